# revision 5
# baseline (speedup 1.0000x reference)
"""Distributed Trainium2 kernel for the AND criterion (retrieval kNN loss).

Math: for L2-normalized rows zn of z [N, d], sim = zn @ zn.T,
logits = sim / T with the diagonal masked, and

  loss_i = -logsumexp_{j in top5}(log_softmax(logits)_ij)
         = log(sum_{j != i} exp(sim_ij/T)) - log(sum_{top5 j != i} exp(sim_ij/T))
  loss   = mean_i loss_i

Only top-5 *values* are needed (exp is monotonic), and the diagonal is always
the row max (sim_ii ~ 1 vs ~0.14 off-diagonal for this data), so per row we
need: the row's top-8 of exp(sim/T) (rank 0 = self, ranks 1..5 = neighbors)
and the full row exp-sum. The DVE max8 instruction gives per-partition top-8
in one pass.

Sharding: rows of z across 8 cores ([1024, 8192] sim block per core); full
z^T is replicated (cheaper than an on-chip all-gather), partial row losses
are summed on host.
"""

import numpy as np
import ml_dtypes
from contextlib import ExitStack

N = 8192
D = 1024
NCORES = 8
LOCAL = N // NCORES          # 1024 rows per core
INV_T = 10.0                 # 1 / temperature
P = 128                      # partitions
K_TILES = D // P             # 8 contraction tiles
M_TILES = LOCAL // P         # 8 output row tiles per core
NC = 512                     # matmul free-dim chunk (one PSUM bank fp32)
N_CHUNKS = N // NC           # 16
L_CHUNKS = LOCAL // NC       # 2
CG = 4                       # sim chunks per PSUM group

_CACHE = {}


def _build():
    import concourse.tile as tile
    import concourse.mybir as mybir
    from concourse import bacc

    dt = mybir.dt
    nc = bacc.Bacc(
        "TRN2", target_bir_lowering=False, debug=False, num_devices=NCORES
    )
    zt_d = nc.dram_tensor("zt", [D, N], dt.bfloat16, kind="ExternalInput")
    zl_d = nc.dram_tensor("zl", [D, LOCAL], dt.bfloat16, kind="ExternalInput")
    out_d = nc.dram_tensor("out", [P, M_TILES], dt.float32, kind="ExternalOutput")

    with tile.TileContext(nc) as tc:
        _body(tc, nc, mybir, zt_d, zl_d, out_d)

    nc.compile()
    return nc


def _body(tc, nc, mybir, zt_d, zl_d, out_d):
    dt = mybir.dt
    AF = mybir.ActivationFunctionType
    AX = mybir.AxisListType

    with ExitStack() as ctx:
        ep = ctx.enter_context
        zt_pool = ep(tc.tile_pool(name="zt", bufs=K_TILES))
        zl_pool = ep(tc.tile_pool(name="zl", bufs=K_TILES))
        const_pool = ep(tc.tile_pool(name="const", bufs=1))
        sq_pool = ep(tc.tile_pool(name="sq", bufs=4))
        rn_pool = ep(tc.tile_pool(name="rn", bufs=4))
        exp_pool = ep(tc.tile_pool(name="exp", bufs=4))
        stat_pool = ep(tc.tile_pool(name="stat", bufs=2))
        small_pool = ep(tc.tile_pool(name="small", bufs=8))
        loss_pool = ep(tc.tile_pool(name="loss", bufs=1))
        psum_pool = ep(tc.tile_pool(name="psum", bufs=8, space="PSUM"))

        ones = const_pool.tile([P, P], dt.bfloat16)
        nc.vector.memset(ones[:], 1.0)

        # ---- load z^T (full, bf16) and local z^T ----
        zt = []
        for k in range(K_TILES):
            t = zt_pool.tile([P, N], dt.bfloat16)
            nc.sync.dma_start(out=t[:], in_=zt_d[k * P:(k + 1) * P, :])
            zt.append(t)
        zl = []
        for k in range(K_TILES):
            t = zl_pool.tile([P, LOCAL], dt.bfloat16)
            nc.sync.dma_start(out=t[:], in_=zl_d[k * P:(k + 1) * P, :])
            zl.append(t)

        # ---- normalize columns of a [d, cols] stack in place ----
        # nrm2 broadcast to all partitions via ones-matmul over the d axis.
        def normalize(tiles, n_cols_chunks):
            for c in range(n_cols_chunks):
                cs = slice(c * NC, (c + 1) * NC)
                ps = psum_pool.tile([P, NC], dt.float32, name="ps", tag="ps")
                for k in range(K_TILES):
                    sq = sq_pool.tile([P, NC], dt.bfloat16)
                    nc.vector.tensor_mul(sq[:], tiles[k][:, cs], tiles[k][:, cs])
                    nc.tensor.matmul(
                        ps[:], lhsT=ones[:], rhs=sq[:],
                        start=(k == 0), stop=(k == K_TILES - 1),
                    )
                rn = rn_pool.tile([P, NC], dt.float32)
                nc.vector.reciprocal(rn[:], ps[:])          # 1 / nrm^2
                rnb = rn_pool.tile([P, NC], dt.bfloat16)
                nc.scalar.activation(rnb[:], rn[:], AF.Sqrt)  # 1 / nrm
                for k in range(K_TILES):
                    nc.vector.tensor_mul(tiles[k][:, cs], tiles[k][:, cs], rnb[:])

        normalize(zt, N_CHUNKS)
        normalize(zl, L_CHUNKS)

        # ---- similarity + row stats ----
        losses = loss_pool.tile([P, M_TILES], dt.float32)
        for mt in range(M_TILES):
            ms = slice(mt * P, (mt + 1) * P)
            sums = stat_pool.tile([P, N_CHUNKS], dt.float32, tag="sums")
            cand = stat_pool.tile([P, N_CHUNKS * 8], dt.float32, tag="cand")
            for cg in range(N_CHUNKS // CG):
                pss = [
                    psum_pool.tile([P, NC], dt.float32, name="ps", tag="ps")
                    for _ in range(CG)
                ]
                for k in range(K_TILES):
                    for ci in range(CG):
                        c = cg * CG + ci
                        nc.tensor.matmul(
                            pss[ci][:],
                            lhsT=zl[k][:, ms],
                            rhs=zt[k][:, c * NC:(c + 1) * NC],
                            start=(k == 0), stop=(k == K_TILES - 1),
                        )
                for ci in range(CG):
                    c = cg * CG + ci
                    ex = exp_pool.tile([P, NC], dt.float32)
                    nc.scalar.activation(
                        ex[:], pss[ci][:], AF.Exp,
                        scale=INV_T, accum_out=sums[:, c:c + 1],
                    )
                    nc.vector.max(out=cand[:, c * 8:(c + 1) * 8], in_=ex[:])

            top8 = small_pool.tile([P, 8], dt.float32, tag="top8")
            nc.vector.max(out=top8[:], in_=cand[:])
            s5 = small_pool.tile([P, 1], dt.float32, tag="s5")
            nc.vector.reduce_sum(s5[:], top8[:, 1:6], mybir.AxisListType.X)
            big = small_pool.tile([P, 1], dt.float32, tag="big")
            nc.vector.reduce_sum(big[:], sums[:], mybir.AxisListType.X)
            sall = small_pool.tile([P, 1], dt.float32, tag="sall")
            nc.vector.tensor_sub(sall[:], big[:], top8[:, 0:1])
            lna = small_pool.tile([P, 1], dt.float32, tag="lna")
            nc.scalar.activation(lna[:], sall[:], AF.Ln)
            ln5 = small_pool.tile([P, 1], dt.float32, tag="ln5")
            nc.scalar.activation(ln5[:], s5[:], AF.Ln)
            nc.vector.tensor_sub(losses[:, mt:mt + 1], lna[:], ln5[:])

        nc.sync.dma_start(out=out_d[:, :], in_=losses[:])


def _get_nc():
    if "nc" not in _CACHE:
        _CACHE["nc"] = _build()
    return _CACHE["nc"]


def _run(z, trace=False):
    from concourse.bass_utils import run_bass_kernel_spmd

    zt = np.ascontiguousarray(z.T).astype(ml_dtypes.bfloat16)  # [D, N]
    in_maps = [
        {"zt": zt, "zl": np.ascontiguousarray(zt[:, i * LOCAL:(i + 1) * LOCAL])}
        for i in range(NCORES)
    ]
    nc = _get_nc()
    res = run_bass_kernel_spmd(
        nc, in_maps, core_ids=list(range(NCORES)), trace=trace
    )
    total = np.float64(0.0)
    for i in range(NCORES):
        total += np.asarray(res.results[i]["out"], dtype=np.float64).sum()
    loss = np.array(total / N, dtype=np.float32)
    return loss, res


def kernel(z):
    loss, _ = _run(np.asarray(z, dtype=np.float32), trace=False)
    return loss


def bench(z, trace=True):
    loss, res = _run(np.asarray(z, dtype=np.float32), trace=trace)
    return loss, res


# revision 6
# speedup vs baseline: 1.1188x; 1.1188x over previous
"""Distributed Trainium2 kernel for the AND criterion (retrieval kNN loss).

Math: for L2-normalized rows zn of z [N, d], sim = zn @ zn.T,
logits = sim / T with the diagonal masked, and

  loss_i = -logsumexp_{j in top5}(log_softmax(logits)_ij)
         = log(sum_{j != i} exp(sim_ij/T)) - log(sum_{top5 j != i} exp(sim_ij/T))
  loss   = mean_i loss_i

Only top-5 *values* are needed (exp is monotonic), and the diagonal is always
the row max (sim_ii ~ 1 vs ~0.14 off-diagonal for this data), so per row we
need: the row's top-8 of exp(sim/T) (rank 0 = self, ranks 1..5 = neighbors)
and the full row exp-sum. The DVE max8 instruction gives per-partition top-8
in one pass.

Sharding: rows of z across 8 cores ([1024, 8192] sim block per core); full
z^T is replicated (cheaper than an on-chip all-gather), partial row losses
are summed on host.
"""

import numpy as np
import ml_dtypes
from contextlib import ExitStack

N = 8192
D = 1024
NCORES = 8
LOCAL = N // NCORES          # 1024 rows per core
INV_T = 10.0                 # 1 / temperature
P = 128                      # partitions
K_TILES = D // P             # 8 contraction tiles
M_TILES = LOCAL // P         # 8 output row tiles per core
NC = 512                     # matmul free-dim chunk (one PSUM bank fp32)
PAIR = 2 * NC                # two banks processed per ACT/DVE epilogue op
N_PAIRS = N // PAIR          # 8
L_PAIRS = LOCAL // PAIR      # 1
DMA_SPLIT = 4                # column-splits per zt row-tile DMA

_CACHE = {}


def _build():
    import concourse.tile as tile
    import concourse.mybir as mybir
    from concourse import bacc

    dt = mybir.dt
    nc = bacc.Bacc(
        "TRN2", target_bir_lowering=False, debug=False, num_devices=NCORES
    )
    zt_d = nc.dram_tensor("zt", [D, N], dt.bfloat16, kind="ExternalInput")
    zl_d = nc.dram_tensor("zl", [D, LOCAL], dt.bfloat16, kind="ExternalInput")
    out_d = nc.dram_tensor("out", [P, M_TILES], dt.float32, kind="ExternalOutput")

    with tile.TileContext(nc) as tc:
        _body(tc, nc, mybir, zt_d, zl_d, out_d)

    nc.compile()
    return nc


def _body(tc, nc, mybir, zt_d, zl_d, out_d):
    dt = mybir.dt
    AF = mybir.ActivationFunctionType
    AX = mybir.AxisListType

    with ExitStack() as ctx:
        ep = ctx.enter_context
        zt_pool = ep(tc.tile_pool(name="zt", bufs=K_TILES))
        zl_pool = ep(tc.tile_pool(name="zl", bufs=K_TILES))
        const_pool = ep(tc.tile_pool(name="const", bufs=1))
        sq_pool = ep(tc.tile_pool(name="sq", bufs=6))
        rn_pool = ep(tc.tile_pool(name="rn", bufs=4))
        exp_pool = ep(tc.tile_pool(name="exp", bufs=4))
        stat_pool = ep(tc.tile_pool(name="stat", bufs=2))
        res_pool = ep(tc.tile_pool(name="res", bufs=1))
        psum_pool = ep(tc.tile_pool(name="psum", bufs=4, space="PSUM"))

        ones = const_pool.tile([P, P], dt.bfloat16)
        nc.vector.memset(ones[:], 1.0)

        # ---- load local z^T first (unblocks the first sim matmuls) ----
        zl = []
        for k in range(K_TILES):
            t = zl_pool.tile([P, LOCAL], dt.bfloat16, name="zlt", tag="zlt")
            nc.sync.dma_start(out=t[:], in_=zl_d[k * P:(k + 1) * P, :])
            zl.append(t)

        # full z^T, column-split DMAs in ascending-chunk order
        zt = [zt_pool.tile([P, N], dt.bfloat16, name="ztt", tag="ztt")
              for _ in range(K_TILES)]
        W = N // DMA_SPLIT
        for s in range(DMA_SPLIT):
            for k in range(K_TILES):
                nc.sync.dma_start(
                    out=zt[k][:, s * W:(s + 1) * W],
                    in_=zt_d[k * P:(k + 1) * P, s * W:(s + 1) * W],
                )

        # ---- normalize columns of a [d, cols] stack in place ----
        # nrm2 broadcast to all partitions via ones-matmul over the d axis;
        # 1/nrm via one ScalarE rsqrt (table loaded before any Exp).
        def normalize(tiles, n_pairs):
            for c in range(n_pairs):
                ps = psum_pool.tile([P, PAIR], dt.float32, name="ps", tag="ps")
                for h in range(2):
                    hs = slice(c * PAIR + h * NC, c * PAIR + (h + 1) * NC)
                    for k in range(K_TILES):
                        sq = sq_pool.tile([P, NC], dt.bfloat16, name="sq", tag="sq")
                        nc.vector.tensor_mul(sq[:], tiles[k][:, hs], tiles[k][:, hs])
                        nc.tensor.matmul(
                            ps[:, h * NC:(h + 1) * NC], lhsT=ones[:], rhs=sq[:],
                            start=(k == 0), stop=(k == K_TILES - 1),
                        )
                rnb = rn_pool.tile([P, PAIR], dt.bfloat16, name="rnb", tag="rnb")
                nc.scalar.activation(rnb[:], ps[:], AF.Abs_reciprocal_sqrt)
                cs = slice(c * PAIR, (c + 1) * PAIR)
                for k in range(K_TILES):
                    nc.vector.tensor_mul(tiles[k][:, cs], tiles[k][:, cs], rnb[:])

        normalize(zl, L_PAIRS)
        normalize(zt, N_PAIRS)

        # ---- similarity + row stats ----
        sall_all = res_pool.tile([P, M_TILES], dt.float32)
        s5_all = res_pool.tile([P, M_TILES], dt.float32)
        for mt in range(M_TILES):
            ms = slice(mt * P, (mt + 1) * P)
            sums = stat_pool.tile([P, N_PAIRS], dt.float32, name="sums", tag="sums")
            cand = stat_pool.tile([P, N_PAIRS * 8], dt.bfloat16, name="cand",
                                  tag="cand")
            for pr in range(N_PAIRS):
                ps = psum_pool.tile([P, PAIR], dt.float32, name="ps", tag="ps")
                for k in range(K_TILES):
                    for h in range(2):
                        cs = slice(pr * PAIR + h * NC, pr * PAIR + (h + 1) * NC)
                        nc.tensor.matmul(
                            ps[:, h * NC:(h + 1) * NC],
                            lhsT=zl[k][:, ms],
                            rhs=zt[k][:, cs],
                            start=(k == 0), stop=(k == K_TILES - 1),
                        )
                ex = exp_pool.tile([P, PAIR], dt.bfloat16, name="ex", tag="ex")
                nc.scalar.activation(
                    ex[:], ps[:], AF.Exp,
                    scale=INV_T, accum_out=sums[:, pr:pr + 1],
                )
                nc.vector.max(out=cand[:, pr * 8:(pr + 1) * 8], in_=ex[:])

            top8 = stat_pool.tile([P, 8], dt.bfloat16, name="top8", tag="top8")
            nc.vector.max(out=top8[:], in_=cand[:])
            nc.vector.reduce_sum(s5_all[:, mt:mt + 1], top8[:, 1:6], AX.X)
            big = stat_pool.tile([P, 1], dt.float32, name="big", tag="big")
            nc.vector.reduce_sum(big[:], sums[:], AX.X)
            nc.vector.tensor_sub(sall_all[:, mt:mt + 1], big[:], top8[:, 0:1])

        # ---- batched logs + output ----
        lna = res_pool.tile([P, M_TILES], dt.float32)
        ln5 = res_pool.tile([P, M_TILES], dt.float32)
        nc.scalar.activation(lna[:], sall_all[:], AF.Ln)
        nc.scalar.activation(ln5[:], s5_all[:], AF.Ln)
        losses = res_pool.tile([P, M_TILES], dt.float32)
        nc.vector.tensor_sub(losses[:], lna[:], ln5[:])
        nc.sync.dma_start(out=out_d[:, :], in_=losses[:])


def _get_nc():
    if "nc" not in _CACHE:
        _CACHE["nc"] = _build()
    return _CACHE["nc"]


def _run(z, trace=False):
    from concourse.bass_utils import run_bass_kernel_spmd

    zt = np.ascontiguousarray(z.T).astype(ml_dtypes.bfloat16)  # [D, N]
    in_maps = [
        {"zt": zt, "zl": np.ascontiguousarray(zt[:, i * LOCAL:(i + 1) * LOCAL])}
        for i in range(NCORES)
    ]
    nc = _get_nc()
    res = run_bass_kernel_spmd(
        nc, in_maps, core_ids=list(range(NCORES)), trace=trace
    )
    total = np.float64(0.0)
    for i in range(NCORES):
        total += np.asarray(res.results[i]["out"], dtype=np.float64).sum()
    loss = np.array(total / N, dtype=np.float32)
    return loss, res


def kernel(z):
    loss, _ = _run(np.asarray(z, dtype=np.float32), trace=False)
    return loss


def bench(z, trace=True):
    loss, res = _run(np.asarray(z, dtype=np.float32), trace=trace)
    return loss, res
